# revision 15
# baseline (speedup 1.0000x reference)
"""ConvNeXt block kernel v8 for Trainium2 (8 NeuronCores, data-parallel).

Block: depthwise 7x7 conv -> LayerNorm(channels) -> MLP 512->2048->512 (exact
GELU) -> layerscale(1e-6) -> residual.  Input x: (16, 512, 32, 32) fp32.

v8 = v6's math, restructured around HW-measured per-matmul costs: every fp8
DoubleRow matmul is LDWEIGHTS-bound (~200-280 ns at FD=512), with an extra
~80 ns penalty when the same stationary is issued twice in a row.  Changes:

  * conv loops are half-OUTER so the 21 diag stationaries rotate every
    matmul (21-link accumulation chains per (ct, half))
  * dx=6 tap column split: Pool does rows 0-15 (half0), DVE rows 16-31,
    for every (img, ct) - halves tap latency, keeps both engines ~60% busy
  * pad copies for BOTH images hoisted before any tap work in the Pool
    queue, so img1's conv never waits on pads
  * out-DMA moved to the DVE queue; the SP queue carries only x-loads and
    prefetches iteration i+1's x during iteration i
  * B phase image-interleaved: MM1(img0), MM1(img1), MM2(img0), MM2(img1);
    MM2 draws PSUM tiles from the (idle in B) conv ring so MM1's ring never
    couples to the epilogue
  * LN-stat matmuls alternate two identical `ones` stationaries at
    different SBUF addresses (same-address back-to-back penalty)

PSUM: ps_conv ring (2x [P,2,512]) serves conv, LN-stat AND MM2 tiles;
ps_mm ring (2x [P,2,512]) serves MM1 ht-pairs.
"""

import sys

if "/opt/trn_rl_repo" not in sys.path:
    sys.path.insert(0, "/opt/trn_rl_repo")

import numpy as np

P = 128
DIM = 512
HID = 2048
CT = DIM // P
HT = HID // P
IMGS = 2
HW = 1024
HALF = 512
PADW = 48
PADH = 39
PADA = PADW * PADH  # 1872 = 16*117
N_CORES = 8
WS = 64.0
MAGIC = 0x5F3759DF

DVE_TAPS = [dy * 7 + 6 for dy in range(7)]  # dx=6 column on Pool/DVE

_cache = {}


def _pairs():
    """21 horizontal tap pairs (dx even, dx+1) for TensorE DoubleRow."""
    return [(dy * 7 + dx, dy * 7 + dx + 1) for dy in range(7) for dx in (0, 2, 4)]


def _np_f8():
    import concourse.mybir as mybir

    return mybir.dt.np(mybir.dt.float8e4)


def _prep(inputs):
    """Host-side parameter folding (model-load-time work)."""
    f8 = _np_f8()
    pairs = _pairs()

    dw_w = np.asarray(inputs["dw_w"], np.float32).reshape(DIM, 49) * WS
    dw_b = np.asarray(inputs["dw_b"], np.float32) * WS
    ln_w = np.asarray(inputs["ln_w"], np.float32)
    ln_b = np.asarray(inputs["ln_b"], np.float32)
    w1 = np.asarray(inputs["w1"], np.float32)
    b1 = np.asarray(inputs["b1"], np.float32)
    w2 = np.asarray(inputs["w2"], np.float32)
    b2 = np.asarray(inputs["b2"], np.float32)
    gam = np.asarray(inputs["gamma"], np.float32)

    def pt(v):
        return np.ascontiguousarray(v.reshape(-1, P).T)

    dw8 = dw_w.astype(f8).astype(np.float32)
    diag = np.zeros((P, len(pairs), CT, 2, P), np.float32)
    idx = np.arange(P)
    for i, (k1, k2) in enumerate(pairs):
        for ct in range(CT):
            for j, k in enumerate((k1, k2)):
                diag[idx, i, ct, j, idx] = dw8[ct * P + idx, k]
    return {
        "diag": np.ascontiguousarray(diag.astype(f8)),
        "wsb": np.ascontiguousarray(dw8.reshape(CT, P, 49).transpose(1, 0, 2)),
        "w1p": np.ascontiguousarray(
            ((ln_w[:, None] * WS) * w1).astype(f8).reshape(CT, P, HID).transpose(1, 0, 2)
        ),
        "w2s": np.ascontiguousarray(
            (w2 * WS).astype(f8).reshape(HT, P, DIM).transpose(1, 0, 2)
        ),
        "bias1v": pt(b1 + w1.T @ ln_b),
        "dwbv": pt(dw_b),
        "gam64v": pt(gam / WS),
        "b2sd": np.ascontiguousarray(
            np.broadcast_to((b2 * WS / 256.0).astype(f8), (P, 2, DIM)).copy()
        ),
    }


def _build(repeat=1, unroll=False):
    import concourse.bass as bass
    import concourse.bacc as bacc
    import concourse.mybir as mybir
    import concourse.tile as tile

    f32 = mybir.dt.float32
    i32 = mybir.dt.int32
    bf16 = mybir.dt.bfloat16
    f8 = mybir.dt.float8e4
    AF = mybir.ActivationFunctionType
    OP = mybir.AluOpType
    PM = mybir.MatmulPerfMode

    nc = bacc.Bacc("TRN2", target_bir_lowering=False)

    pairs = _pairs()
    n_pairs = len(pairs)

    x_d = nc.dram_tensor("x", (IMGS, DIM, 32, 32), f32, kind="ExternalInput")
    diag_d = nc.dram_tensor("diag", (P, n_pairs, CT, 2, P), f8, kind="ExternalInput")
    wsb_d = nc.dram_tensor("wsb", (P, CT, 49), f32, kind="ExternalInput")
    w1p_d = nc.dram_tensor("w1p", (P, CT, HID), f8, kind="ExternalInput")
    w2s_d = nc.dram_tensor("w2s", (P, HT, DIM), f8, kind="ExternalInput")
    bias1_d = nc.dram_tensor("bias1v", (P, HT), f32, kind="ExternalInput")
    dwb_d = nc.dram_tensor("dwbv", (P, CT), f32, kind="ExternalInput")
    gam64_d = nc.dram_tensor("gam64v", (P, CT), f32, kind="ExternalInput")
    b2sd_d = nc.dram_tensor("b2sd", (P, 2, DIM), f8, kind="ExternalInput")
    out_d = nc.dram_tensor("out", (IMGS, DIM, 32, 32), f32, kind="ExternalOutput")

    with tile.TileContext(nc) as tc:
        from contextlib import ExitStack

        with ExitStack() as ctx:
            consts = ctx.enter_context(tc.tile_pool(name="consts", bufs=1))
            xpool = ctx.enter_context(tc.tile_pool(name="xpool", bufs=2))
            ypool = ctx.enter_context(tc.tile_pool(name="ypool", bufs=2))
            sqpool = ctx.enter_context(tc.tile_pool(name="sqpool", bufs=2))
            lnt = ctx.enter_context(tc.tile_pool(name="lnt", bufs=4))
            lnt2 = ctx.enter_context(tc.tile_pool(name="lnt2", bufs=2))
            yppool = ctx.enter_context(tc.tile_pool(name="yppool", bufs=4))
            gpool = ctx.enter_context(tc.tile_pool(name="gpool", bufs=4))
            opool = ctx.enter_context(tc.tile_pool(name="opool", bufs=2))
            ps_conv = ctx.enter_context(
                tc.tile_pool(name="ps_conv", bufs=2, space="PSUM")
            )
            ps_mm = ctx.enter_context(tc.tile_pool(name="ps_mm", bufs=2, space="PSUM"))

            # ---------------- constants ----------------
            dwb = consts.tile([P, CT], f32)
            nc.sync.dma_start(dwb, dwb_d[:])
            gam64 = consts.tile([P, CT], f32)
            nc.sync.dma_start(gam64, gam64_d[:])
            b2sd = consts.tile([P, 2, DIM], f8)
            nc.sync.dma_start(b2sd, b2sd_d[:])
            bias1 = consts.tile([P, HT], f32)
            nc.sync.dma_start(bias1, bias1_d[:])
            w_sb = consts.tile([P, CT, 49], f32)
            nc.sync.dma_start(w_sb, wsb_d[:])
            diag = consts.tile([P, n_pairs, CT, 2, P], f8)
            for ct in range(CT):
                nc.scalar.dma_start(diag[:, :, ct, :, :], diag_d[:][:, :, ct, :, :])
            w1p = consts.tile([P, CT, HID], f8)
            nc.scalar.dma_start(w1p, w1p_d[:])
            w2s = consts.tile([P, HT, DIM], f8)
            nc.scalar.dma_start(w2s, w2s_d[:])

            ones_mov = consts.tile([P, 2, HALF], f8)
            nc.vector.memset(ones_mov, 1.0)
            # two identical stat stationaries at different addresses: rotate
            # to dodge the same-address back-to-back LDWEIGHTS penalty
            ones2 = consts.tile([P, 2, 2, P], f8)
            nc.vector.memset(ones2, 1.0 / 64.0)

            padf_all = consts.tile([P, IMGS, CT, 2, PADA], f8)
            nc.vector.memset(padf_all, 0.0)

            # ---------------- steady-state loop ----------------
            # repeat>1: hardware For_i loop (timing runs). unroll=True emits
            # the body `repeat` times in python instead (TimelineSim can't
            # resolve reg-mode branches).
            rep_cm = None
            n_emit = 1
            if repeat > 1:
                if unroll:
                    n_emit = repeat
                else:
                    rep_cm = tc.For_i(0, repeat, 1)
                    rep_cm.__enter__()

            for _emit in range(n_emit):
                x_sbs, ys, sqs, vfgs = [], [], [], []
                stats = {}

                # ---- A0: x loads (SP queue only) + pad copies (Pool) ----
                for img in range(IMGS):
                    x_sb = xpool.tile([P, CT, HW], f32, tag="x")
                    xdr = x_d[:][img].rearrange("(t p) h w -> p t (h w)", p=P)
                    for ct in range(CT):
                        nc.sync.dma_start(x_sb[:, ct, :], xdr[:, ct, :])
                    x_sbs.append(x_sb)
                for img in range(IMGS):
                    x_sb = x_sbs[img]
                    padf = padf_all[:, img]
                    vfg_img = []
                    for ct in range(CT):
                        vf = padf[:, ct, 0, :]
                        xi = x_sb[:, ct, :].rearrange("p (h w) -> p h w", h=32)
                        vfg = vf[:, : PADW * PADH].rearrange(
                            "p (r c) -> p r c", r=PADH
                        )
                        nc.gpsimd.tensor_copy(vfg[:, 3:35, 3:35], xi)
                        vf2g = padf[:, ct, 1, : PADW * PADH].rearrange(
                            "p (r c) -> p r c", r=PADH
                        )
                        nc.gpsimd.tensor_copy(vf2g[:, 3:35, 2:34], xi)
                        vfg_img.append(vfg)
                    vfgs.append(vfg_img)

                # ---- A1: conv chains (PE) + merge (Act) + taps (Pool/DVE)
                #      + sq (Act), per image ----
                for img in range(IMGS):
                    y = ypool.tile([P, CT, HW], f8, tag="y")
                    ys.append(y)
                    padf = padf_all[:, img]

                    for ct in range(CT):
                        vf = padf[:, ct, 0, :]
                        cps = ps_conv.tile([P, 2, HALF], f32, tag="cps")
                        for half in range(2):
                            for i, (k1, k2) in enumerate(pairs):
                                dy, dx = divmod(k1, 7)
                                off = (16 * half + dy) * PADW + dx
                                base = vf[:, off : off + 1]
                                rhs = bass.AP(
                                    tensor=base.tensor,
                                    offset=base.offset,
                                    ap=[
                                        list(base.ap[0]),
                                        [PADA, 2],
                                        [PADW, 16],
                                        [1, 32],
                                    ],
                                )
                                nc.tensor.matmul(
                                    cps[:, half, :].rearrange(
                                        "p (r c) -> p r c", r=16
                                    ),
                                    diag[:, i, ct, :, :],
                                    rhs,
                                    start=(i == 0),
                                    stop=(i == n_pairs - 1),
                                    perf_mode=PM.DoubleRow,
                                    skip_group_check=True,
                                )

                        nc.scalar.activation(
                            y[:, ct, :],
                            cps.rearrange("p h n -> p (h n)"),
                            AF.Identity,
                            bias=dwb[:, ct : ct + 1],
                            scale=1.0,
                        )
                        # dx=6 tap column on VectorE (Pool lacks the
                        # scalar-pointer stt opcode)
                        y3 = y[:, ct, :].rearrange("p (r c) -> p r c", c=32)
                        vfg = vfgs[img][ct]
                        for k in DVE_TAPS:
                            dy, dx = divmod(k, 7)
                            nc.vector.scalar_tensor_tensor(
                                y3,
                                vfg[:, dy : dy + 32, dx : dx + 32],
                                w_sb[:, ct, k : k + 1],
                                y3,
                                OP.mult,
                                OP.add,
                            )

                    sq = sqpool.tile([P, CT, HW], f8, tag="sq")
                    sqs.append(sq)
                    for ct in range(CT):
                        nc.scalar.activation(
                            sq[:, ct, :], y[:, ct, :], AF.Square, scale=0.125
                        )

                # ---- S phase: LN stats + early drain ----
                for img in range(IMGS):
                    y, sq = ys[img], sqs[img]
                    for half in range(2):
                        sl = slice(half * HALF, (half + 1) * HALF)
                        st = ps_conv.tile([P, 2, HALF], f32, tag="cps", name="st")
                        for j in range(CT // 2):
                            nc.tensor.matmul(
                                st[:, 0, :], ones2[:, j, :, :],
                                y[:, 2 * j : 2 * j + 2, sl],
                                start=(j == 0), stop=(j == CT // 2 - 1),
                                perf_mode=PM.DoubleRow, skip_group_check=True,
                            )
                        for j in range(CT // 2):
                            nc.tensor.matmul(
                                st[:, 1, :], ones2[:, j, :, :],
                                sq[:, 2 * j : 2 * j + 2, sl],
                                start=(j == 0), stop=(j == CT // 2 - 1),
                                perf_mode=PM.DoubleRow, skip_group_check=True,
                            )
                        mu_bf = lnt.tile([P, HALF], bf16, tag="mubf")
                        nc.scalar.copy(mu_bf, st[:, 0, :])
                        mu2 = lnt2.tile([P, HALF], f32, tag="mu2")
                        nc.scalar.activation(mu2, st[:, 0, :], AF.Square, scale=0.125)
                        var = lnt.tile([P, HALF], f32, tag="var")
                        nc.vector.scalar_tensor_tensor(
                            var, st[:, 1, :], 8.0, mu2, OP.mult, OP.subtract
                        )
                        stats[(img, half)] = (mu_bf, var)

                # ---- B phase: normalize (DVE), then MM1 img0, MM1 img1,
                #      MM2 img0, MM2 img1 ----
                yps, gs = {}, {}
                for img in range(IMGS):
                    y = ys[img]
                    for half in range(2):
                        sl = slice(half * HALF, (half + 1) * HALF)
                        mu_bf, var = stats[(img, half)]

                        r_ = lnt.tile([P, HALF], f32, tag="r")
                        t_ = lnt2.tile([P, HALF], i32, tag="ti")
                        nc.vector.tensor_scalar(
                            t_, var.bitcast(i32), -1, 1,
                            OP.bitwise_xor, OP.arith_shift_right,
                        )
                        nc.vector.tensor_scalar(
                            r_.bitcast(i32), t_, MAGIC + 1, None, OP.add
                        )

                        yp = yppool.tile([P, CT, HALF], f8, tag="yp")
                        for ct in range(CT):
                            td = lnt2.tile([P, HALF], bf16, tag="td")
                            nc.vector.scalar_tensor_tensor(
                                td, mu_bf, -0.125, y[:, ct, sl], OP.mult, OP.add
                            )
                            nc.vector.tensor_mul(yp[:, ct, :], td, r_)
                        yps[(img, half)] = yp

                for img in range(IMGS):
                    for half in range(2):
                        yp = yps[(img, half)]
                        g = gpool.tile([P, HT, HALF], f8, tag="g")
                        gs[(img, half)] = g
                        for hp in range(HT // 2):
                            ps = ps_mm.tile([P, 2, HALF], f32, tag="mm")
                            for h2 in range(2):
                                ht = 2 * hp + h2
                                for j in range(CT // 2):
                                    nc.tensor.matmul(
                                        ps[:, h2, :],
                                        w1p[
                                            :, 2 * j : 2 * j + 2,
                                            ht * P : (ht + 1) * P,
                                        ],
                                        yp[:, 2 * j : 2 * j + 2, :],
                                        start=(j == 0),
                                        stop=(j == CT // 2 - 1),
                                        perf_mode=PM.DoubleRow,
                                        skip_group_check=True,
                                    )
                            for h2 in range(2):
                                ht = 2 * hp + h2
                                nc.scalar.activation(
                                    g[:, ht, :], ps[:, h2, :], AF.Gelu,
                                    bias=bias1[:, ht : ht + 1], scale=1.0 / WS,
                                )

                for img in range(IMGS):
                    x_sb = x_sbs[img]
                    for half in range(2):
                        sl = slice(half * HALF, (half + 1) * HALF)
                        g = gs[(img, half)]
                        for cp in range(CT // 2):
                            o = opool.tile([P, 2, HALF], f32, tag="o")
                            ps2 = ps_conv.tile(
                                [P, 2, HALF], f32, tag="cps", name="ps2"
                            )
                            for c2 in range(2):
                                ct = 2 * cp + c2
                                nc.tensor.matmul(
                                    ps2[:, c2, :],
                                    b2sd[:, :, ct * P : (ct + 1) * P],
                                    ones_mov,
                                    start=True, stop=False,
                                    perf_mode=PM.DoubleRow, skip_group_check=True,
                                )
                                for j in range(HT // 2):
                                    nc.tensor.matmul(
                                        ps2[:, c2, :],
                                        w2s[
                                            :, 2 * j : 2 * j + 2,
                                            ct * P : (ct + 1) * P,
                                        ],
                                        g[:, 2 * j : 2 * j + 2, :],
                                        start=False,
                                        stop=(j == HT // 2 - 1),
                                        perf_mode=PM.DoubleRow,
                                        skip_group_check=True,
                                    )
                            nc.vector.scalar_tensor_tensor(
                                o,
                                ps2,
                                gam64[:, 2 * cp : 2 * cp + 1],
                                x_sb[:, 2 * cp : 2 * cp + 2, sl],
                                OP.mult,
                                OP.add,
                            )
                            nc.sync.dma_start(
                                out_d[:][img].rearrange(
                                    "(t p) h w -> p t (h w)", p=P
                                )[:, 2 * cp : 2 * cp + 2, sl],
                                o,
                            )

            if rep_cm is not None:
                rep_cm.__exit__(None, None, None)

    nc.compile()
    return nc


def _get_nc(repeat=1):
    key = ("nc", repeat)
    if key not in _cache:
        _cache[key] = _build(repeat)
    return _cache[key]


def run(inputs, trace=False, repeat=1, cores=N_CORES, **kw):
    from concourse.bass_utils import run_bass_kernel_spmd

    nc = _get_nc(repeat)
    prep = _prep(inputs)
    x = np.ascontiguousarray(np.asarray(inputs["x"], np.float32))
    in_maps = []
    for core in range(cores):
        m = dict(prep)
        m["x"] = np.ascontiguousarray(
            x[(core * IMGS) % 16 : (core * IMGS) % 16 + IMGS]
        )
        in_maps.append(m)
    res = run_bass_kernel_spmd(
        nc, in_maps, core_ids=list(range(cores)), trace=trace, **kw
    )
    out = np.concatenate([r["out"] for r in res.results], axis=0)
    return out, res


def kernel(**inputs):
    out, _ = run(inputs)
    return out


# revision 22
# speedup vs baseline: 1.0389x; 1.0389x over previous
"""ConvNeXt block kernel v8 for Trainium2 (8 NeuronCores, data-parallel).

Block: depthwise 7x7 conv -> LayerNorm(channels) -> MLP 512->2048->512 (exact
GELU) -> layerscale(1e-6) -> residual.  Input x: (16, 512, 32, 32) fp32.
Each core takes 2 images; conv runs as fp8 DoubleRow diagonal matmuls on the
PE (21 horizontal tap pairs via two shifted pad copies), the dx=6 tap column
on VectorE, LN stats as fp8 ones-matmuls, the MLP as fp8 DoubleRow GEMMs.

v8 = v6's math, restructured around HW-measured per-matmul costs: every fp8
DoubleRow matmul at FD=512 is LDWEIGHTS-bound (~200-280 ns), with an extra
~80 ns penalty when the same stationary is issued twice in a row.  Changes
vs v6:

  * conv loops are half-OUTER so the 21 diag stationaries rotate every
    matmul (21-link accumulation chains per (ct, half))
  * pad copies for BOTH images hoisted to the head of the Pool queue, so
    img1's conv never waits on pads
  * the SP queue carries x-loads first and the out-stores after, letting
    iteration i+1's x prefetch during iteration i's B phase
  * B phase image-interleaved: MM1(img0) both halves, then MM1(img1)
    interleaved with ungated MM2(img0) cp-chains (PE keeps streaming while
    the Act GELU backlog drains), then MM2(img1); MM2 draws PSUM tiles from
    the conv ring (idle in B) so MM1's ring never couples to the epilogue
  * MM2's 16 bias matmuls are gone: gamma*b2 is folded into the residual
    input on VectorE after the pad copies consume x (exact math)
  * LN-stat matmuls alternate two identical `ones` stationaries at
    different SBUF addresses (same-address back-to-back penalty)

PSUM: ps_conv ring (2x [P,2,512]) serves conv, LN-stat AND MM2 tiles;
ps_mm ring (2x [P,2,512]) serves MM1 ht-pairs.
"""

import sys

if "/opt/trn_rl_repo" not in sys.path:
    sys.path.insert(0, "/opt/trn_rl_repo")

import numpy as np

P = 128
DIM = 512
HID = 2048
CT = DIM // P
HT = HID // P
IMGS = 2
HW = 1024
HALF = 512
PADW = 48
PADH = 39
PADA = PADW * PADH  # 1872 = 16*117
N_CORES = 8
WS = 64.0
MAGIC = 0x5F3759DF

DVE_TAPS = [dy * 7 + 6 for dy in range(7)]  # dx=6 column on Pool/DVE

_cache = {}


def _pairs():
    """21 horizontal tap pairs (dx even, dx+1) for TensorE DoubleRow."""
    return [(dy * 7 + dx, dy * 7 + dx + 1) for dy in range(7) for dx in (0, 2, 4)]


def _np_f8():
    import concourse.mybir as mybir

    return mybir.dt.np(mybir.dt.float8e4)


def _prep(inputs):
    """Host-side parameter folding (model-load-time work)."""
    f8 = _np_f8()
    pairs = _pairs()

    dw_w = np.asarray(inputs["dw_w"], np.float32).reshape(DIM, 49) * WS
    dw_b = np.asarray(inputs["dw_b"], np.float32) * WS
    ln_w = np.asarray(inputs["ln_w"], np.float32)
    ln_b = np.asarray(inputs["ln_b"], np.float32)
    w1 = np.asarray(inputs["w1"], np.float32)
    b1 = np.asarray(inputs["b1"], np.float32)
    w2 = np.asarray(inputs["w2"], np.float32)
    b2 = np.asarray(inputs["b2"], np.float32)
    gam = np.asarray(inputs["gamma"], np.float32)

    def pt(v):
        return np.ascontiguousarray(v.reshape(-1, P).T)

    dw8 = dw_w.astype(f8).astype(np.float32)
    diag = np.zeros((P, len(pairs), CT, 2, P), np.float32)
    idx = np.arange(P)
    for i, (k1, k2) in enumerate(pairs):
        for ct in range(CT):
            for j, k in enumerate((k1, k2)):
                diag[idx, i, ct, j, idx] = dw8[ct * P + idx, k]
    return {
        "diag": np.ascontiguousarray(diag.astype(f8)),
        "wsb": np.ascontiguousarray(dw8.reshape(CT, P, 49).transpose(1, 0, 2)),
        "w1p": np.ascontiguousarray(
            ((ln_w[:, None] * WS) * w1).astype(f8).reshape(CT, P, HID).transpose(1, 0, 2)
        ),
        "w2s": np.ascontiguousarray(
            (w2 * WS).astype(f8).reshape(HT, P, DIM).transpose(1, 0, 2)
        ),
        "bias1v": pt(b1 + w1.T @ ln_b),
        "dwbv": pt(dw_b),
        "gam64v": pt(gam / WS),
        "gb2v": pt(gam * b2),
    }


def _build(repeat=1, unroll=False):
    import concourse.bass as bass
    import concourse.bacc as bacc
    import concourse.mybir as mybir
    import concourse.tile as tile

    f32 = mybir.dt.float32
    i32 = mybir.dt.int32
    bf16 = mybir.dt.bfloat16
    f8 = mybir.dt.float8e4
    AF = mybir.ActivationFunctionType
    OP = mybir.AluOpType
    PM = mybir.MatmulPerfMode

    nc = bacc.Bacc("TRN2", target_bir_lowering=False)

    pairs = _pairs()
    n_pairs = len(pairs)

    x_d = nc.dram_tensor("x", (IMGS, DIM, 32, 32), f32, kind="ExternalInput")
    diag_d = nc.dram_tensor("diag", (P, n_pairs, CT, 2, P), f8, kind="ExternalInput")
    wsb_d = nc.dram_tensor("wsb", (P, CT, 49), f32, kind="ExternalInput")
    w1p_d = nc.dram_tensor("w1p", (P, CT, HID), f8, kind="ExternalInput")
    w2s_d = nc.dram_tensor("w2s", (P, HT, DIM), f8, kind="ExternalInput")
    bias1_d = nc.dram_tensor("bias1v", (P, HT), f32, kind="ExternalInput")
    dwb_d = nc.dram_tensor("dwbv", (P, CT), f32, kind="ExternalInput")
    gam64_d = nc.dram_tensor("gam64v", (P, CT), f32, kind="ExternalInput")
    gb2_d = nc.dram_tensor("gb2v", (P, CT), f32, kind="ExternalInput")
    out_d = nc.dram_tensor("out", (IMGS, DIM, 32, 32), f32, kind="ExternalOutput")

    with tile.TileContext(nc) as tc:
        from contextlib import ExitStack

        with ExitStack() as ctx:
            consts = ctx.enter_context(tc.tile_pool(name="consts", bufs=1))
            xpool = ctx.enter_context(tc.tile_pool(name="xpool", bufs=2))
            ypool = ctx.enter_context(tc.tile_pool(name="ypool", bufs=2))
            sqpool = ctx.enter_context(tc.tile_pool(name="sqpool", bufs=2))
            lnt = ctx.enter_context(tc.tile_pool(name="lnt", bufs=4))
            lnt2 = ctx.enter_context(tc.tile_pool(name="lnt2", bufs=2))
            yppool = ctx.enter_context(tc.tile_pool(name="yppool", bufs=4))
            gpool = ctx.enter_context(tc.tile_pool(name="gpool", bufs=4))
            opool = ctx.enter_context(tc.tile_pool(name="opool", bufs=2))
            ps_conv = ctx.enter_context(
                tc.tile_pool(name="ps_conv", bufs=2, space="PSUM")
            )
            ps_mm = ctx.enter_context(tc.tile_pool(name="ps_mm", bufs=2, space="PSUM"))

            # ---------------- constants ----------------
            dwb = consts.tile([P, CT], f32)
            nc.sync.dma_start(dwb, dwb_d[:])
            gam64 = consts.tile([P, CT], f32)
            nc.sync.dma_start(gam64, gam64_d[:])
            gb2 = consts.tile([P, CT], f32)
            nc.sync.dma_start(gb2, gb2_d[:])
            bias1 = consts.tile([P, HT], f32)
            nc.sync.dma_start(bias1, bias1_d[:])
            w_sb = consts.tile([P, CT, 49], f32)
            nc.sync.dma_start(w_sb, wsb_d[:])
            diag = consts.tile([P, n_pairs, CT, 2, P], f8)
            for ct in range(CT):
                nc.scalar.dma_start(diag[:, :, ct, :, :], diag_d[:][:, :, ct, :, :])
            w1p = consts.tile([P, CT, HID], f8)
            nc.scalar.dma_start(w1p, w1p_d[:])
            w2s = consts.tile([P, HT, DIM], f8)
            nc.scalar.dma_start(w2s, w2s_d[:])

            # two identical stat stationaries at different addresses: rotate
            # to dodge the same-address back-to-back LDWEIGHTS penalty
            ones2 = consts.tile([P, 2, 2, P], f8)
            nc.vector.memset(ones2, 1.0 / 64.0)

            padf_all = consts.tile([P, IMGS, CT, 2, PADA], f8)
            nc.vector.memset(padf_all, 0.0)

            # ---------------- steady-state loop ----------------
            # repeat>1: hardware For_i loop (timing runs). unroll=True emits
            # the body `repeat` times in python instead (TimelineSim can't
            # resolve reg-mode branches).
            rep_cm = None
            n_emit = 1
            if repeat > 1:
                if unroll:
                    n_emit = repeat
                else:
                    rep_cm = tc.For_i(0, repeat, 1)
                    rep_cm.__enter__()

            for _emit in range(n_emit):
                x_sbs, ys, sqs, vfgs = [], [], [], []

                # ---- A0: x loads (SP queue only) + pad copies (Pool) ----
                for img in range(IMGS):
                    x_sb = xpool.tile([P, CT, HW], f32, tag="x")
                    xdr = x_d[:][img].rearrange("(t p) h w -> p t (h w)", p=P)
                    for ct in range(CT):
                        nc.sync.dma_start(x_sb[:, ct, :], xdr[:, ct, :])
                    x_sbs.append(x_sb)
                for img in range(IMGS):
                    x_sb = x_sbs[img]
                    padf = padf_all[:, img]
                    vfg_img = []
                    for ct in range(CT):
                        vf = padf[:, ct, 0, :]
                        xi = x_sb[:, ct, :].rearrange("p (h w) -> p h w", h=32)
                        vfg = vf[:, : PADW * PADH].rearrange(
                            "p (r c) -> p r c", r=PADH
                        )
                        nc.gpsimd.tensor_copy(vfg[:, 3:35, 3:35], xi)
                        vf2g = padf[:, ct, 1, : PADW * PADH].rearrange(
                            "p (r c) -> p r c", r=PADH
                        )
                        nc.gpsimd.tensor_copy(vf2g[:, 3:35, 2:34], xi)
                        vfg_img.append(vfg)
                    vfgs.append(vfg_img)

                # ---- A1: conv chains (PE) + merge (Act) + taps (Pool/DVE)
                #      + sq (Act), per image ----
                for img in range(IMGS):
                    y = ypool.tile([P, CT, HW], f8, tag="y")
                    ys.append(y)
                    padf = padf_all[:, img]

                    for ct in range(CT):
                        vf = padf[:, ct, 0, :]
                        cps = ps_conv.tile([P, 2, HALF], f32, tag="cps")
                        for half in range(2):
                            for i, (k1, k2) in enumerate(pairs):
                                dy, dx = divmod(k1, 7)
                                off = (16 * half + dy) * PADW + dx
                                base = vf[:, off : off + 1]
                                rhs = bass.AP(
                                    tensor=base.tensor,
                                    offset=base.offset,
                                    ap=[
                                        list(base.ap[0]),
                                        [PADA, 2],
                                        [PADW, 16],
                                        [1, 32],
                                    ],
                                )
                                nc.tensor.matmul(
                                    cps[:, half, :].rearrange(
                                        "p (r c) -> p r c", r=16
                                    ),
                                    diag[:, i, ct, :, :],
                                    rhs,
                                    start=(i == 0),
                                    stop=(i == n_pairs - 1),
                                    perf_mode=PM.DoubleRow,
                                    skip_group_check=True,
                                )

                        nc.scalar.activation(
                            y[:, ct, :],
                            cps.rearrange("p h n -> p (h n)"),
                            AF.Identity,
                            bias=dwb[:, ct : ct + 1],
                            scale=1.0,
                        )
                        # dx=6 tap column on VectorE (Pool lacks the
                        # scalar-pointer stt opcode)
                        y3 = y[:, ct, :].rearrange("p (r c) -> p r c", c=32)
                        vfg = vfgs[img][ct]
                        for k in DVE_TAPS:
                            dy, dx = divmod(k, 7)
                            nc.vector.scalar_tensor_tensor(
                                y3,
                                vfg[:, dy : dy + 32, dx : dx + 32],
                                w_sb[:, ct, k : k + 1],
                                y3,
                                OP.mult,
                                OP.add,
                            )

                    sq = sqpool.tile([P, CT, HW], f8, tag="sq")
                    sqs.append(sq)
                    for cq in range(CT // 2):
                        nc.scalar.activation(
                            sq[:, 2 * cq : 2 * cq + 2, :],
                            y[:, 2 * cq : 2 * cq + 2, :],
                            AF.Square, scale=0.125,
                        )
                    # fold gamma*b2 into the residual input (pads already
                    # consumed x, epilogue adds x_sb, so this lands the MM2
                    # bias without a PE bias matmul)
                    x_sb = x_sbs[img]
                    for ct in range(CT):
                        nc.vector.tensor_scalar(
                            x_sb[:, ct, :], x_sb[:, ct, :],
                            gb2[:, ct : ct + 1], None, OP.add,
                        )

                # ---- S phase + normalize, per image: LN stats, early
                # drain, then rsqrt+normalize immediately so norm(img0)
                # isn't queued behind var(img1) on the DVE ----
                yps, gs = {}, {}
                for img in range(IMGS):
                    y, sq = ys[img], sqs[img]
                    for half in range(2):
                        sl = slice(half * HALF, (half + 1) * HALF)
                        st = ps_conv.tile([P, 2, HALF], f32, tag="cps", name="st")
                        for j in range(CT // 2):
                            nc.tensor.matmul(
                                st[:, 0, :], ones2[:, j, :, :],
                                y[:, 2 * j : 2 * j + 2, sl],
                                start=(j == 0), stop=(j == CT // 2 - 1),
                                perf_mode=PM.DoubleRow, skip_group_check=True,
                            )
                        for j in range(CT // 2):
                            nc.tensor.matmul(
                                st[:, 1, :], ones2[:, j, :, :],
                                sq[:, 2 * j : 2 * j + 2, sl],
                                start=(j == 0), stop=(j == CT // 2 - 1),
                                perf_mode=PM.DoubleRow, skip_group_check=True,
                            )
                        mu_bf = lnt.tile([P, HALF], bf16, tag="mubf")
                        nc.scalar.copy(mu_bf, st[:, 0, :])
                        mu2 = lnt2.tile([P, HALF], f32, tag="mu2")
                        nc.scalar.activation(mu2, st[:, 0, :], AF.Square, scale=0.125)
                        var = lnt.tile([P, HALF], f32, tag="var")
                        nc.vector.scalar_tensor_tensor(
                            var, st[:, 1, :], 8.0, mu2, OP.mult, OP.subtract
                        )

                        r_ = lnt.tile([P, HALF], f32, tag="r")
                        t_ = lnt2.tile([P, HALF], i32, tag="ti")
                        nc.vector.tensor_scalar(
                            t_, var.bitcast(i32), -1, 1,
                            OP.bitwise_xor, OP.arith_shift_right,
                        )
                        nc.vector.tensor_scalar(
                            r_.bitcast(i32), t_, MAGIC + 1, None, OP.add
                        )

                        yp = yppool.tile([P, CT, HALF], f8, tag="yp")
                        for ct in range(CT):
                            td = lnt2.tile([P, HALF], bf16, tag="td")
                            nc.vector.scalar_tensor_tensor(
                                td, mu_bf, -0.125, y[:, ct, sl], OP.mult, OP.add
                            )
                            nc.vector.tensor_mul(yp[:, ct, :], td, r_)
                        yps[(img, half)] = yp

                def mm1_steps(img, half):
                    yp = yps[(img, half)]
                    g = gpool.tile([P, HT, HALF], f8, tag="g")
                    gs[(img, half)] = g

                    def step(hp):
                        ps = ps_mm.tile([P, 2, HALF], f32, tag="mm")
                        for h2 in range(2):
                            ht = 2 * hp + h2
                            for j in range(CT // 2):
                                nc.tensor.matmul(
                                    ps[:, h2, :],
                                    w1p[
                                        :, 2 * j : 2 * j + 2,
                                        ht * P : (ht + 1) * P,
                                    ],
                                    yp[:, 2 * j : 2 * j + 2, :],
                                    start=(j == 0),
                                    stop=(j == CT // 2 - 1),
                                    perf_mode=PM.DoubleRow,
                                    skip_group_check=True,
                                )
                        for h2 in range(2):
                            ht = 2 * hp + h2
                            nc.scalar.activation(
                                g[:, ht, :], ps[:, h2, :], AF.Gelu,
                                bias=bias1[:, ht : ht + 1], scale=1.0 / WS,
                            )

                    return [lambda hp=hp: step(hp) for hp in range(HT // 2)]

                def mm2_steps(img, half):
                    x_sb = x_sbs[img]
                    sl = slice(half * HALF, (half + 1) * HALF)

                    def step(cp):
                        g = gs[(img, half)]
                        o = opool.tile([P, 2, HALF], f32, tag="o")
                        ps2 = ps_conv.tile(
                            [P, 2, HALF], f32, tag="cps", name="ps2"
                        )
                        for c2 in range(2):
                            ct = 2 * cp + c2
                            for j in range(HT // 2):
                                nc.tensor.matmul(
                                    ps2[:, c2, :],
                                    w2s[
                                        :, 2 * j : 2 * j + 2,
                                        ct * P : (ct + 1) * P,
                                    ],
                                    g[:, 2 * j : 2 * j + 2, :],
                                    start=(j == 0),
                                    stop=(j == HT // 2 - 1),
                                    perf_mode=PM.DoubleRow,
                                    skip_group_check=True,
                                )
                        nc.vector.scalar_tensor_tensor(
                            o,
                            ps2,
                            gam64[:, 2 * cp : 2 * cp + 1],
                            x_sb[:, 2 * cp : 2 * cp + 2, sl],
                            OP.mult,
                            OP.add,
                        )
                        nc.sync.dma_start(
                            out_d[:][img].rearrange(
                                "(t p) h w -> p t (h w)", p=P
                            )[:, 2 * cp : 2 * cp + 2, sl],
                            o,
                        )

                    return [lambda cp=cp: step(cp) for cp in range(CT // 2)]

                def interleave(mm1, mm2):
                    # lead with an ungated MM2 cp-chain so the PE streams
                    # while the Act GELU backlog drains, then alternate with
                    # the Act-gated MM1 hp-groups
                    order = mm2[:1] + mm1[:4] + mm2[1:] + mm1[4:]
                    for s in order:
                        s()

                # MM1(img0) both halves first so GELU(img0) completes early
                for s in mm1_steps(0, 0):
                    s()
                for s in mm1_steps(0, 1):
                    s()
                # MM1(img1) interleaved with MM2(img0)
                interleave(mm1_steps(1, 0), mm2_steps(0, 0))
                interleave(mm1_steps(1, 1), mm2_steps(0, 1))
                for s in mm2_steps(1, 0):
                    s()
                for s in mm2_steps(1, 1):
                    s()

            if rep_cm is not None:
                rep_cm.__exit__(None, None, None)

    nc.compile()
    return nc


def _get_nc(repeat=1):
    key = ("nc", repeat)
    if key not in _cache:
        _cache[key] = _build(repeat)
    return _cache[key]


def run(inputs, trace=False, repeat=1, cores=N_CORES, **kw):
    from concourse.bass_utils import run_bass_kernel_spmd

    nc = _get_nc(repeat)
    prep = _prep(inputs)
    x = np.ascontiguousarray(np.asarray(inputs["x"], np.float32))
    in_maps = []
    for core in range(cores):
        m = dict(prep)
        m["x"] = np.ascontiguousarray(
            x[(core * IMGS) % 16 : (core * IMGS) % 16 + IMGS]
        )
        in_maps.append(m)
    res = run_bass_kernel_spmd(
        nc, in_maps, core_ids=list(range(cores)), trace=trace, **kw
    )
    out = np.concatenate([r["out"] for r in res.results], axis=0)
    return out, res


def kernel(**inputs):
    out, _ = run(inputs)
    return out


# revision 29
# speedup vs baseline: 1.1773x; 1.1332x over previous
"""ConvNeXt block kernel v10 for Trainium2 (8 NeuronCores, data-parallel).

Block: depthwise 7x7 conv -> LayerNorm(channels) -> MLP 512->2048->512 (exact
GELU) -> layerscale(1e-6) -> residual.  Input x: (16, 512, 32, 32) fp32.
Each core takes 2 images; conv runs as fp8 DoubleRow diagonal matmuls on the
PE (21 horizontal tap pairs via two shifted pad copies), the dx=6 tap column
on VectorE, LN stats as fp8 ones-matmuls, the MLP as fp8 DoubleRow GEMMs.

v8 = v6's math, restructured around HW-measured per-matmul costs: every fp8
DoubleRow matmul at FD=512 is LDWEIGHTS-bound (~200-280 ns), with an extra
~80 ns penalty when the same stationary is issued twice in a row.  Changes
vs v6:

  * conv loops are half-OUTER so the 21 diag stationaries rotate every
    matmul (21-link accumulation chains per (ct, half))
  * pad copies for BOTH images hoisted to the head of the Pool queue, so
    img1's conv never waits on pads
  * the SP queue carries x-loads first and the out-stores after, letting
    iteration i+1's x prefetch during iteration i's B phase
  * B phase image-interleaved: MM1(img0) both halves, then MM1(img1)
    interleaved with ungated MM2(img0) cp-chains (PE keeps streaming while
    the Act GELU backlog drains), then MM2(img1); MM2 draws PSUM tiles from
    the conv ring (idle in B) so MM1's ring never couples to the epilogue
  * MM2's 16 bias matmuls are gone: gamma*b2 is folded into the residual
    input on VectorE after the pad copies consume x (exact math); emitted
    after S+normalize so those DVE ops never delay the norm chain
  * LN-stat matmuls alternate two identical `ones` stationaries at
    different SBUF addresses (same-address back-to-back penalty)
  * S phase and normalize are fused per image, so norm(img0) runs on the
    DVE right after var(img0) instead of queueing behind var(img1) - this
    pulls MM1(img0)'s start earlier (measured ~11%); furthermore
    stats+norm(img0) is spliced into the middle of conv(img1)'s ct loop so
    the whole img0 LN drain chain (Act mu ops, DVE var/rsqrt/normalize)
    runs during conv(img1) instead of on the critical path before MM1
  * conv merge + dx=6 taps run per HALF, so each ct's DVE tap chain starts
    as soon as that half's 21-link chain stops (tap tail gates S/B start)
  * MM1 PSUM tiles are 1-bank [P,512] in a 4-deep ring (same 4-bank
    budget): finer GELU granularity, PE runs further ahead of Act

PSUM: ps_conv ring (2x [P,2,512]) serves conv, LN-stat AND MM2 tiles;
ps_mm ring (4x [P,512]) serves MM1 ht chains.
"""

import sys

if "/opt/trn_rl_repo" not in sys.path:
    sys.path.insert(0, "/opt/trn_rl_repo")

import numpy as np

P = 128
DIM = 512
HID = 2048
CT = DIM // P
HT = HID // P
IMGS = 2
HW = 1024
HALF = 512
PADW = 48
PADH = 39
PADA = PADW * PADH  # 1872 = 16*117
N_CORES = 8
WS = 64.0
MAGIC = 0x5F3759DF

DVE_TAPS = [dy * 7 + 6 for dy in range(7)]  # dx=6 column on Pool/DVE

_cache = {}


def _pairs():
    """21 horizontal tap pairs (dx even, dx+1) for TensorE DoubleRow."""
    return [(dy * 7 + dx, dy * 7 + dx + 1) for dy in range(7) for dx in (0, 2, 4)]


def _np_f8():
    import concourse.mybir as mybir

    return mybir.dt.np(mybir.dt.float8e4)


def _prep(inputs):
    """Host-side parameter folding (model-load-time work)."""
    f8 = _np_f8()
    pairs = _pairs()

    dw_w = np.asarray(inputs["dw_w"], np.float32).reshape(DIM, 49) * WS
    dw_b = np.asarray(inputs["dw_b"], np.float32) * WS
    ln_w = np.asarray(inputs["ln_w"], np.float32)
    ln_b = np.asarray(inputs["ln_b"], np.float32)
    w1 = np.asarray(inputs["w1"], np.float32)
    b1 = np.asarray(inputs["b1"], np.float32)
    w2 = np.asarray(inputs["w2"], np.float32)
    b2 = np.asarray(inputs["b2"], np.float32)
    gam = np.asarray(inputs["gamma"], np.float32)

    def pt(v):
        return np.ascontiguousarray(v.reshape(-1, P).T)

    dw8 = dw_w.astype(f8).astype(np.float32)
    diag = np.zeros((P, len(pairs), CT, 2, P), np.float32)
    idx = np.arange(P)
    for i, (k1, k2) in enumerate(pairs):
        for ct in range(CT):
            for j, k in enumerate((k1, k2)):
                diag[idx, i, ct, j, idx] = dw8[ct * P + idx, k]
    return {
        "diag": np.ascontiguousarray(diag.astype(f8)),
        "wsb": np.ascontiguousarray(dw8.reshape(CT, P, 49).transpose(1, 0, 2)),
        "w1p": np.ascontiguousarray(
            ((ln_w[:, None] * WS) * w1).astype(f8).reshape(CT, P, HID).transpose(1, 0, 2)
        ),
        "w2s": np.ascontiguousarray(
            (w2 * WS).astype(f8).reshape(HT, P, DIM).transpose(1, 0, 2)
        ),
        "bias1v": pt(b1 + w1.T @ ln_b),
        "dwbv": pt(dw_b),
        "gam64v": pt(gam / WS),
        "gb2v": pt(gam * b2),
    }


def _build(repeat=1, unroll=False):
    import concourse.bass as bass
    import concourse.bacc as bacc
    import concourse.mybir as mybir
    import concourse.tile as tile

    f32 = mybir.dt.float32
    i32 = mybir.dt.int32
    bf16 = mybir.dt.bfloat16
    f8 = mybir.dt.float8e4
    AF = mybir.ActivationFunctionType
    OP = mybir.AluOpType
    PM = mybir.MatmulPerfMode

    nc = bacc.Bacc("TRN2", target_bir_lowering=False)

    pairs = _pairs()
    n_pairs = len(pairs)

    x_d = nc.dram_tensor("x", (IMGS, DIM, 32, 32), f32, kind="ExternalInput")
    diag_d = nc.dram_tensor("diag", (P, n_pairs, CT, 2, P), f8, kind="ExternalInput")
    wsb_d = nc.dram_tensor("wsb", (P, CT, 49), f32, kind="ExternalInput")
    w1p_d = nc.dram_tensor("w1p", (P, CT, HID), f8, kind="ExternalInput")
    w2s_d = nc.dram_tensor("w2s", (P, HT, DIM), f8, kind="ExternalInput")
    bias1_d = nc.dram_tensor("bias1v", (P, HT), f32, kind="ExternalInput")
    dwb_d = nc.dram_tensor("dwbv", (P, CT), f32, kind="ExternalInput")
    gam64_d = nc.dram_tensor("gam64v", (P, CT), f32, kind="ExternalInput")
    gb2_d = nc.dram_tensor("gb2v", (P, CT), f32, kind="ExternalInput")
    out_d = nc.dram_tensor("out", (IMGS, DIM, 32, 32), f32, kind="ExternalOutput")

    with tile.TileContext(nc) as tc:
        from contextlib import ExitStack

        with ExitStack() as ctx:
            consts = ctx.enter_context(tc.tile_pool(name="consts", bufs=1))
            xpool = ctx.enter_context(tc.tile_pool(name="xpool", bufs=2))
            ypool = ctx.enter_context(tc.tile_pool(name="ypool", bufs=2))
            sqpool = ctx.enter_context(tc.tile_pool(name="sqpool", bufs=2))
            lnt = ctx.enter_context(tc.tile_pool(name="lnt", bufs=4))
            lnt2 = ctx.enter_context(tc.tile_pool(name="lnt2", bufs=2))
            yppool = ctx.enter_context(tc.tile_pool(name="yppool", bufs=4))
            gpool = ctx.enter_context(tc.tile_pool(name="gpool", bufs=4))
            opool = ctx.enter_context(tc.tile_pool(name="opool", bufs=2))
            ps_conv = ctx.enter_context(
                tc.tile_pool(name="ps_conv", bufs=2, space="PSUM")
            )
            ps_mm = ctx.enter_context(tc.tile_pool(name="ps_mm", bufs=4, space="PSUM"))

            # ---------------- constants ----------------
            dwb = consts.tile([P, CT], f32)
            nc.sync.dma_start(dwb, dwb_d[:])
            gam64 = consts.tile([P, CT], f32)
            nc.sync.dma_start(gam64, gam64_d[:])
            gb2 = consts.tile([P, CT], f32)
            nc.sync.dma_start(gb2, gb2_d[:])
            bias1 = consts.tile([P, HT], f32)
            nc.sync.dma_start(bias1, bias1_d[:])
            w_sb = consts.tile([P, CT, 49], f32)
            nc.sync.dma_start(w_sb, wsb_d[:])
            diag = consts.tile([P, n_pairs, CT, 2, P], f8)
            for ct in range(CT):
                nc.scalar.dma_start(diag[:, :, ct, :, :], diag_d[:][:, :, ct, :, :])
            w1p = consts.tile([P, CT, HID], f8)
            nc.scalar.dma_start(w1p, w1p_d[:])
            w2s = consts.tile([P, HT, DIM], f8)
            nc.scalar.dma_start(w2s, w2s_d[:])

            # two identical stat stationaries at different addresses: rotate
            # to dodge the same-address back-to-back LDWEIGHTS penalty
            ones2 = consts.tile([P, 2, 2, P], f8)
            nc.vector.memset(ones2, 1.0 / 64.0)

            padf_all = consts.tile([P, IMGS, CT, 2, PADA], f8)
            nc.vector.memset(padf_all, 0.0)

            # ---------------- steady-state loop ----------------
            # repeat>1: hardware For_i loop (timing runs). unroll=True emits
            # the body `repeat` times in python instead (TimelineSim can't
            # resolve reg-mode branches).
            rep_cm = None
            n_emit = 1
            if repeat > 1:
                if unroll:
                    n_emit = repeat
                else:
                    rep_cm = tc.For_i(0, repeat, 1)
                    rep_cm.__enter__()

            for _emit in range(n_emit):
                x_sbs, ys, sqs, vfgs = [], [], [], []

                # ---- A0: x loads (SP queue only) + pad copies (Pool) ----
                for img in range(IMGS):
                    x_sb = xpool.tile([P, CT, HW], f32, tag="x")
                    xdr = x_d[:][img].rearrange("(t p) h w -> p t (h w)", p=P)
                    for ct in range(CT):
                        nc.sync.dma_start(x_sb[:, ct, :], xdr[:, ct, :])
                    x_sbs.append(x_sb)
                for img in range(IMGS):
                    x_sb = x_sbs[img]
                    padf = padf_all[:, img]
                    vfg_img = []
                    for ct in range(CT):
                        vf = padf[:, ct, 0, :]
                        xi = x_sb[:, ct, :].rearrange("p (h w) -> p h w", h=32)
                        vfg = vf[:, : PADW * PADH].rearrange(
                            "p (r c) -> p r c", r=PADH
                        )
                        nc.gpsimd.tensor_copy(vfg[:, 3:35, 3:35], xi)
                        vf2g = padf[:, ct, 1, : PADW * PADH].rearrange(
                            "p (r c) -> p r c", r=PADH
                        )
                        nc.gpsimd.tensor_copy(vf2g[:, 3:35, 2:34], xi)
                        vfg_img.append(vfg)
                    vfgs.append(vfg_img)

                yps, gs = {}, {}

                def emit_s_norm(img):
                    # LN stats, early drain, rsqrt+normalize for one
                    # image.  img0 is spliced into the middle of
                    # conv(img1)'s ct loop so its Act/DVE drain chain
                    # runs during conv(img1) instead of on the
                    # critical path before MM1.
                    y, sq = ys[img], sqs[img]
                    for half in range(2):
                        sl = slice(half * HALF, (half + 1) * HALF)
                        st = ps_conv.tile([P, 2, HALF], f32, tag="cps", name="st")
                        for j in range(CT // 2):
                            nc.tensor.matmul(
                                st[:, 0, :], ones2[:, j, :, :],
                                y[:, 2 * j : 2 * j + 2, sl],
                                start=(j == 0), stop=(j == CT // 2 - 1),
                                perf_mode=PM.DoubleRow, skip_group_check=True,
                            )
                        for j in range(CT // 2):
                            nc.tensor.matmul(
                                st[:, 1, :], ones2[:, j, :, :],
                                sq[:, 2 * j : 2 * j + 2, sl],
                                start=(j == 0), stop=(j == CT // 2 - 1),
                                perf_mode=PM.DoubleRow, skip_group_check=True,
                            )
                        mu_bf = lnt.tile([P, HALF], bf16, tag="mubf")
                        nc.scalar.copy(mu_bf, st[:, 0, :])
                        mu2 = lnt2.tile([P, HALF], f32, tag="mu2")
                        nc.scalar.activation(mu2, st[:, 0, :], AF.Square, scale=0.125)
                        var = lnt.tile([P, HALF], f32, tag="var")
                        nc.vector.scalar_tensor_tensor(
                            var, st[:, 1, :], 8.0, mu2, OP.mult, OP.subtract
                        )

                        r_ = lnt.tile([P, HALF], f32, tag="r")
                        t_ = lnt2.tile([P, HALF], i32, tag="ti")
                        nc.vector.tensor_scalar(
                            t_, var.bitcast(i32), -1, 1,
                            OP.bitwise_xor, OP.arith_shift_right,
                        )
                        nc.vector.tensor_scalar(
                            r_.bitcast(i32), t_, MAGIC + 1, None, OP.add
                        )

                        yp = yppool.tile([P, CT, HALF], f8, tag="yp")
                        for ct in range(CT):
                            td = lnt2.tile([P, HALF], bf16, tag="td")
                            nc.vector.scalar_tensor_tensor(
                                td, mu_bf, -0.125, y[:, ct, sl], OP.mult, OP.add
                            )
                            nc.vector.tensor_mul(yp[:, ct, :], td, r_)
                        yps[(img, half)] = yp

                # ---- A1: conv chains (PE) + merge (Act) + taps (Pool/DVE)
                #      + sq (Act), per image ----
                for img in range(IMGS):
                    y = ypool.tile([P, CT, HW], f8, tag="y")
                    ys.append(y)
                    padf = padf_all[:, img]

                    for ct in range(CT):
                        if img == 1 and ct == 2:
                            emit_s_norm(0)
                        vf = padf[:, ct, 0, :]
                        vfg = vfgs[img][ct]
                        cps = ps_conv.tile([P, 2, HALF], f32, tag="cps")
                        for half in range(2):
                            for i, (k1, k2) in enumerate(pairs):
                                dy, dx = divmod(k1, 7)
                                off = (16 * half + dy) * PADW + dx
                                base = vf[:, off : off + 1]
                                rhs = bass.AP(
                                    tensor=base.tensor,
                                    offset=base.offset,
                                    ap=[
                                        list(base.ap[0]),
                                        [PADA, 2],
                                        [PADW, 16],
                                        [1, 32],
                                    ],
                                )
                                nc.tensor.matmul(
                                    cps[:, half, :].rearrange(
                                        "p (r c) -> p r c", r=16
                                    ),
                                    diag[:, i, ct, :, :],
                                    rhs,
                                    start=(i == 0),
                                    stop=(i == n_pairs - 1),
                                    perf_mode=PM.DoubleRow,
                                    skip_group_check=True,
                                )
                            # per-half merge + dx=6 taps: the DVE tap chain
                            # for this half starts as soon as its conv chain
                            # stops, ~half a ct earlier than a full-ct merge
                            hs = slice(half * HALF, (half + 1) * HALF)
                            nc.scalar.activation(
                                y[:, ct, hs],
                                cps[:, half, :],
                                AF.Identity,
                                bias=dwb[:, ct : ct + 1],
                                scale=1.0,
                            )
                            y3h = y[:, ct, hs].rearrange(
                                "p (r c) -> p r c", c=32
                            )
                            for k in DVE_TAPS:
                                dy, dx = divmod(k, 7)
                                nc.vector.scalar_tensor_tensor(
                                    y3h,
                                    vfg[
                                        :, dy + 16 * half : dy + 16 * half + 16,
                                        dx : dx + 32,
                                    ],
                                    w_sb[:, ct, k : k + 1],
                                    y3h,
                                    OP.mult,
                                    OP.add,
                                )

                    sq = sqpool.tile([P, CT, HW], f8, tag="sq")
                    sqs.append(sq)
                    for cq in range(CT // 2):
                        nc.scalar.activation(
                            sq[:, 2 * cq : 2 * cq + 2, :],
                            y[:, 2 * cq : 2 * cq + 2, :],
                            AF.Square, scale=0.125,
                        )
                emit_s_norm(1)

                # fold gamma*b2 into the residual input (pads consumed x
                # long ago; only the late-B epilogue reads x_sb) - emitted
                # here so these DVE ops queue AFTER the normalize, not
                # before it
                for img in range(IMGS):
                    x_sb = x_sbs[img]
                    for ct in range(CT):
                        nc.vector.tensor_scalar(
                            x_sb[:, ct, :], x_sb[:, ct, :],
                            gb2[:, ct : ct + 1], None, OP.add,
                        )

                def mm1_steps(img, half):
                    yp = yps[(img, half)]
                    g = gpool.tile([P, HT, HALF], f8, tag="g")
                    gs[(img, half)] = g

                    def step(ht):
                        ps = ps_mm.tile([P, HALF], f32, tag="mm")
                        for j in range(CT // 2):
                            nc.tensor.matmul(
                                ps,
                                w1p[
                                    :, 2 * j : 2 * j + 2,
                                    ht * P : (ht + 1) * P,
                                ],
                                yp[:, 2 * j : 2 * j + 2, :],
                                start=(j == 0),
                                stop=(j == CT // 2 - 1),
                                perf_mode=PM.DoubleRow,
                                skip_group_check=True,
                            )
                        nc.scalar.activation(
                            g[:, ht, :], ps, AF.Gelu,
                            bias=bias1[:, ht : ht + 1], scale=1.0 / WS,
                        )

                    return [lambda ht=ht: step(ht) for ht in range(HT)]

                def mm2_steps(img, half):
                    x_sb = x_sbs[img]
                    sl = slice(half * HALF, (half + 1) * HALF)

                    def step(cp):
                        g = gs[(img, half)]
                        o = opool.tile([P, 2, HALF], f32, tag="o")
                        ps2 = ps_conv.tile(
                            [P, 2, HALF], f32, tag="cps", name="ps2"
                        )
                        for c2 in range(2):
                            ct = 2 * cp + c2
                            for j in range(HT // 2):
                                nc.tensor.matmul(
                                    ps2[:, c2, :],
                                    w2s[
                                        :, 2 * j : 2 * j + 2,
                                        ct * P : (ct + 1) * P,
                                    ],
                                    g[:, 2 * j : 2 * j + 2, :],
                                    start=(j == 0),
                                    stop=(j == HT // 2 - 1),
                                    perf_mode=PM.DoubleRow,
                                    skip_group_check=True,
                                )
                        nc.vector.scalar_tensor_tensor(
                            o,
                            ps2,
                            gam64[:, 2 * cp : 2 * cp + 1],
                            x_sb[:, 2 * cp : 2 * cp + 2, sl],
                            OP.mult,
                            OP.add,
                        )
                        nc.sync.dma_start(
                            out_d[:][img].rearrange(
                                "(t p) h w -> p t (h w)", p=P
                            )[:, 2 * cp : 2 * cp + 2, sl],
                            o,
                        )

                    return [lambda cp=cp: step(cp) for cp in range(CT // 2)]

                def interleave(mm1, mm2):
                    # lead with an ungated MM2 cp-chain so the PE streams
                    # while the Act GELU backlog drains, then alternate with
                    # the Act-gated MM1 hp-groups
                    order = mm2[:1] + mm1[:8] + mm2[1:] + mm1[8:]
                    for s in order:
                        s()

                # MM1(img0) both halves first so GELU(img0) completes early
                for s in mm1_steps(0, 0):
                    s()
                for s in mm1_steps(0, 1):
                    s()
                # MM1(img1) interleaved with MM2(img0)
                interleave(mm1_steps(1, 0), mm2_steps(0, 0))
                interleave(mm1_steps(1, 1), mm2_steps(0, 1))
                for s in mm2_steps(1, 0):
                    s()
                for s in mm2_steps(1, 1):
                    s()

            if rep_cm is not None:
                rep_cm.__exit__(None, None, None)

    nc.compile()
    return nc


def _get_nc(repeat=1):
    key = ("nc", repeat)
    if key not in _cache:
        _cache[key] = _build(repeat)
    return _cache[key]


def run(inputs, trace=False, repeat=1, cores=N_CORES, **kw):
    from concourse.bass_utils import run_bass_kernel_spmd

    nc = _get_nc(repeat)
    prep = _prep(inputs)
    x = np.ascontiguousarray(np.asarray(inputs["x"], np.float32))
    in_maps = []
    for core in range(cores):
        m = dict(prep)
        m["x"] = np.ascontiguousarray(
            x[(core * IMGS) % 16 : (core * IMGS) % 16 + IMGS]
        )
        in_maps.append(m)
    res = run_bass_kernel_spmd(
        nc, in_maps, core_ids=list(range(cores)), trace=trace, **kw
    )
    out = np.concatenate([r["out"] for r in res.results], axis=0)
    return out, res


def kernel(**inputs):
    out, _ = run(inputs)
    return out
